# revision 50
# baseline (speedup 1.0000x reference)
"""Multi-head causal attention (B=2, S=2048, D=1024, H=16) on 8 trn2 cores.

Sharding: core c -> (batch b = c//4, head-group g = c%4, 4 heads each).
Data-parallel over B, tensor-parallel over heads. Each core computes a
partial output projection [S, D] in bf16; the host sums the 4 partials
per batch and adds b_out.

Device kernel per core:
  A) qk projection in fp8e4 with DoubleRow matmuls (kt-pairs folded in
     the free dim, 0.5 cyc/row); v projection in bf16 (fp8 v fails the
     2e-2 gate). qkT layout [128, 4, S] fp8: partition 32h+p = head h
     dim p; f-tiles 0/1 = K dims 0-31/32-63, 2/3 = Q — so scores run as
     DoubleRow with the two 32-dim halves folded. Biases via per-
     partition DVE tensor_scalar_add on the PSUM->fp8 copy (qk) and a
     rank-1 matmul (v).
  B) per head h, per 512-wide query block qmb: causal flash attention
     in the scores-TRANSPOSED layout: sT[k,q] = K @ Q^T (fp8 DoubleRow)
     so attn@V is lhsT=v_blk[s,hd+1] bf16 (ones col -> softmax denoms
     in PSUM row 64), rhs=expT[k,q] bf16. Causal mask on the PE via a
     DoubleRow fp8 matmul [I|0].T @ [M|M] with M in {0,-240}. The
     sc->exp->av chains are software-pipelined (av emission lags by
     `lag` links) through rotating PSUM pools; A-half1 items drip into
     B(qmb0/1) as PE filler with staged drains at qmb2/qmb3.
  C) out_partial[s, 1024] = values^T.T @ W_out in bf16, DMA'd to DRAM.
  x is prefetched across repeat iterations: half-0 tiles are loaded by
  the prologue and reloaded mid-body for the next iteration.
"""
import math
import numpy as np

import concourse.bass as bass
import concourse.mybir as mybir
import concourse.tile as tile
from concourse import bacc
from concourse.bass_utils import run_bass_kernel_spmd

N_CORES = 8
B, S, D = 2, 2048, 1024
H = 16                    # total heads
HL = 4                    # heads per core
HD = 64                   # head dim
FQK = 2 * HL * HD         # 512 local q+k features
FV = HL * HD              # 256 local v features
SCALE = 1.0 / math.sqrt(HD)
NEG = -240.0              # fp8e4-representable; exp(-240/8) ~ 9e-14

QMB = 512                 # query macro-block
KB = 128                  # key block
N_QMB = S // QMB          # 4
N_KB = S // KB            # 16

F32 = mybir.dt.float32
F32R = mybir.dt.float32r
BF16 = mybir.dt.bfloat16
FP8 = mybir.dt.float8e4
DR = mybir.MatmulPerfMode.DoubleRow


def build_kernel(repeat: int = 1, stages: str = "ABC", xdma_in_loop: bool = True,
                 bmode: str = "full", pairw: int = 2, wave: int = 2,
                 sc_bufs: int = 3, av_bufs: int = 2, exp_bufs: int = 12,
                 lag: int = 4, fullexp: bool = False,
                 fill_first: bool = False):
    assert sc_bufs * pairw + av_bufs <= 8
    assert wave == 2, "only the fp8-DoubleRow wave=2 path is implemented"
    W = 512 * pairw
    nc = bacc.Bacc(
        "TRN2", target_bir_lowering=False, debug=False, num_devices=N_CORES
    )
    xT = nc.dram_tensor("xT", [D, S], BF16, kind="ExternalInput")
    xT8 = nc.dram_tensor("xT8", [D, S], FP8, kind="ExternalInput")
    wqk = nc.dram_tensor("wqk", [D, FQK], FP8, kind="ExternalInput")
    wv = nc.dram_tensor("wv", [D, FV], BF16, kind="ExternalInput")
    wo = nc.dram_tensor("wo", [FV, D], BF16, kind="ExternalInput")
    bqk = nc.dram_tensor("bqk", [FQK], F32, kind="ExternalInput")
    bv = nc.dram_tensor("bv", [FV], F32R, kind="ExternalInput")
    out = nc.dram_tensor("out", [S, D], BF16, kind="ExternalOutput")

    KT = D // 128  # 8 contraction tiles over D

    with tile.TileContext(nc) as tc:
        dma = nc.sync  # HWDGE: spreads transfers over HW queues
        with (
            tc.tile_pool(name="const", bufs=1) as const,
            tc.tile_pool(name="xt", bufs=1) as xtp,
            tc.tile_pool(name="big", bufs=1) as big,
            tc.tile_pool(name="exp", bufs=exp_bufs) as expp,
            tc.tile_pool(name="small", bufs=4) as small,
            tc.tile_pool(name="ob", bufs=3) as obp,
            tc.tile_pool(name="ps_sc", bufs=sc_bufs, space="PSUM") as ps_sc,
            tc.tile_pool(name="ps_av", bufs=av_bufs, space="PSUM") as ps_av,
        ):
            # ---- constants ----
            wqk_sb = const.tile([128, KT, FQK], FP8)
            wv_sb = const.tile([128, KT, FV], BF16)
            wo_sb = const.tile([128, FV // 128, D], BF16)
            dma.dma_start(
                out=wqk_sb, in_=wqk.rearrange("(kt p) f -> p kt f", p=128)
            )
            dma.dma_start(
                out=wv_sb, in_=wv.rearrange("(kt p) f -> p kt f", p=128)
            )
            dma.dma_start(
                out=wo_sb, in_=wo.rearrange("(dt p) f -> p dt f", p=128)
            )
            # qk bias as per-partition scalars: bqk_sb[p, ft]
            bqk_sb = const.tile([128, 4], F32)
            bv_sb = const.tile([1, FV], F32R)
            dma.dma_start(out=bqk_sb, in_=bqk.rearrange("(t p) -> p t", p=128))
            dma.dma_start(out=bv_sb, in_=bv.rearrange("(o f) -> o f", o=1))
            ones_f32 = const.tile([1, QMB], F32)
            nc.vector.memset(ones_f32, 1.0)
            ones_row = const.tile([1, QMB], F32R)
            nc.vector.tensor_copy(ones_row, ones_f32)
            # additive causal mask for the diagonal 128x128 block:
            # trimask[k, q] = 0 if k <= q else NEG, applied on the PE via a
            # DoubleRow fp8 matmul: [I | 0].T @ [M | M] = M
            trimask = const.tile([128, 128], F32)
            nc.gpsimd.memset(trimask, 0.0)
            nc.gpsimd.affine_select(
                out=trimask,
                in_=trimask,
                compare_op=mybir.AluOpType.is_ge,
                fill=NEG,
                base=0,
                pattern=[[1, 128]],
                channel_multiplier=-1,
            )
            trimask_dr = const.tile([128, 2, 128], FP8)
            nc.vector.tensor_copy(trimask_dr[:, 0, :], trimask)
            nc.vector.tensor_copy(trimask_dr[:, 1, :], trimask)
            ident_f32 = const.tile([128, 128], F32)
            nc.gpsimd.memset(ident_f32, 0.0)
            nc.gpsimd.affine_select(
                out=ident_f32,
                in_=ident_f32,
                compare_op=mybir.AluOpType.not_equal,
                fill=1.0,
                base=0,
                pattern=[[-1, 128]],
                channel_multiplier=1,
            )
            identz_dr = const.tile([128, 2, 128], FP8)
            nc.vector.tensor_copy(identz_dr[:, 0, :], ident_f32)
            nc.vector.memset(identz_dr[:, 1, :], 0.0)

            # ---- persistent intermediates ----
            # qkT: fp8 for DoubleRow scores. partition 32h+p = head h, dim p;
            # f-tiles: 0 = K dims 0-31, 1 = K dims 32-63, 2 = Q lo, 3 = Q hi.
            qkT = big.tile([128, 4, S], FP8)
            v_aug = big.tile([128, N_KB, HL, HD + 1], BF16)
            valuesT = big.tile([128, FV // 128, S], BF16)
            vone_f32 = const.tile([128, N_KB * HL], F32)
            nc.vector.memset(vone_f32, 1.0)
            nc.vector.tensor_copy(
                v_aug[:, :, :, HD:HD + 1],
                vone_f32.rearrange("p (kb h o) -> p kb h o", h=HL, o=1),
            )

            # ---- x tiles: per-half buffers enabling cross-iteration
            # prefetch. Half 0 is loaded by the prologue (iter 0) and
            # RELOADED mid-body for the next iteration; half 1 loads at
            # body start and is consumed by the A-half1 filler items.
            xT8_r = xT8.rearrange("(t p) s -> p t s", p=128)
            xt_tiles = {
                (half, kt): xtp.tile([128, S // 2], BF16,
                                     tag=f"xt{half}_{kt}",
                                     name=f"xt{half}_{kt}")
                for half in range(2) for kt in range(KT)
            }
            x8_tiles = {
                (half, kp): xtp.tile([128, 2, S // 2], FP8,
                                     tag=f"x8{half}_{kp}",
                                     name=f"x8{half}_{kp}")
                for half in range(2) for kp in range(KT // 2)
            }

            def emit_x_loads(half):
                s0 = half * (S // 2)
                for kp in range(KT // 2):
                    dma.dma_start(
                        out=x8_tiles[(half, kp)],
                        in_=xT8_r[:, 2 * kp:2 * kp + 2, s0:s0 + S // 2],
                    )
                for kt in range(KT):
                    dma.dma_start(
                        out=xt_tiles[(half, kt)],
                        in_=xT[kt * 128:(kt + 1) * 128, s0:s0 + S // 2],
                    )

            emit_x_loads(0)  # prologue: iteration 0's half-0 data

            def body(_it):
                # ======== stage A: qkT and v_aug, in two column halves ====
                # Half 0 runs up front; half 1 is queued as PE-filler work
                # items dripped into B(qmb0/1), which only need half 0.
                def load_xts(half):
                    return [xt_tiles[(half, kt)] for kt in range(KT)]

                def load_xts8(half):
                    return [x8_tiles[(half, kp)] for kp in range(KT // 2)]

                def make_qk_item(xts8, half, ft, nt):
                    s0 = half * (S // 2)

                    def emit():
                        c0 = nt * 512
                        ps = ps_sc.tile([128, W], F32, tag="sc")
                        for kp in range(KT // 2):
                            nc.tensor.matmul(
                                ps[:, 0:512],
                                wqk_sb[:, 2 * kp:2 * kp + 2,
                                       ft * 128:(ft + 1) * 128],
                                xts8[kp][:, :, c0:c0 + 512],
                                start=(kp == 0),
                                stop=(kp == KT // 2 - 1),
                                perf_mode=DR,
                            )
                        # bias add (per-partition) fused into the PSUM->fp8 copy
                        nc.vector.tensor_scalar_add(
                            qkT[:, ft, s0 + c0:s0 + c0 + 512],
                            ps[:, 0:512],
                            bqk_sb[:, ft:ft + 1],
                        )
                    return emit

                def make_v_item(xts, half, stp):
                    def emit():
                        psv = ps_sc.tile([128, 512], F32, tag="sc")
                        for sub in range(2):
                            sti = stp * 2 + sub
                            c0 = sub * FV
                            for kt in range(KT):
                                nc.tensor.matmul(
                                    psv[:, c0:c0 + FV],
                                    xts[kt][:, sti * 128:(sti + 1) * 128],
                                    wv_sb[:, kt, :],
                                    start=(kt == 0),
                                    stop=False,
                                )
                            nc.tensor.matmul(
                                psv[:, c0:c0 + FV],
                                ones_row[0:1, 0:128],
                                bv_sb,
                                start=False,
                                stop=True,
                            )
                        st0 = half * 8 + stp * 2
                        nc.vector.tensor_copy(
                            v_aug[:, st0:st0 + 2, :, 0:HD],
                            psv.rearrange("s (t h c) -> s t h c", t=2, h=HL),
                        )
                    return emit

                def a_items(xts, xts8, half):
                    items = []
                    for ft in range(4):
                        for nt in range(2):
                            items.append(make_qk_item(xts8, half, ft, nt))
                    for stp in range(4):
                        items.append(make_v_item(xts, half, stp))
                    return items

                # half-0 data was loaded by the prologue (iter 0) or by the
                # previous iteration's mid-body reload.
                for it in a_items(load_xts(0), load_xts8(0), 0):
                    it()
                emit_x_loads(1)
                # A-half1 as PE filler dripped into B(qmb0/1); ordered so the
                # six items qmb2 needs (qk nt=0, v stp0/1) come first.
                h1 = a_items(load_xts(1), load_xts8(1), 1)
                filler = [h1[i] for i in (0, 2, 4, 6, 8, 9, 1, 3, 5, 7, 10, 11)]
                a_set = set(filler)
                n_late = 6  # trailing items only needed by qmb3
                # prefetch next iteration's half-0 x while B runs (WAR on
                # this iteration's A-half0 readers, which are already done)
                emit_x_loads(0)

                if "B" not in stages:
                    # sink so DCE keeps stage A
                    dma.dma_start(
                        out=out[0:128, 0:512],
                        in_=qkT[:, 0, 0:512].bitcast(F32),
                    )
                    return

                # ======== stage B+C: per query macro-block ========
                # A-half1 and C work are drip-fed into B's matmul stream
                # as PE filler (keeps the PE HAM-warm).

                def make_c_item(st):
                    def emit():
                        ob = obp.tile([128, 1024], BF16)
                        for nt in range(2):
                            ps = ps_sc.tile([128, W], F32, tag="sc")
                            for dt_ in range(FV // 128):
                                nc.tensor.matmul(
                                    ps[:, 0:512],
                                    valuesT[:, dt_, st * 128:(st + 1) * 128],
                                    wo_sb[:, dt_, nt * 512:(nt + 1) * 512],
                                    start=(dt_ == 0),
                                    stop=(dt_ == FV // 128 - 1),
                                )
                            nc.vector.tensor_copy(
                                ob[:, nt * 512:(nt + 1) * 512], ps[:, 0:512]
                            )
                        dma.dma_start(
                            out=out[st * 128:(st + 1) * 128, :], in_=ob
                        )
                    return emit

                for qmb in range(N_QMB):
                    if qmb == 2:
                        # qmb2 needs the essential half-1 items; the late six
                        # (plus any C backlog) keep dripping through its blks
                        while len([f for f in filler if f in a_set]) > n_late:
                            filler.pop(0)()
                    if qmb == 3:
                        while any(f in a_set for f in filler):
                            filler.pop(0)()
                    q0 = qmb * QMB
                    nkb = 4 * qmb + 4
                    nblk = nkb // pairw
                    for w0 in range(0, HL, wave):
                        whs = list(range(w0, w0 + wave))
                        avs = {
                            h_: ps_av.tile([65, QMB], F32, tag="av",
                                           name=f"av{h_}")
                            for h_ in whs
                        }
                        avq = []

                        def emit_av(item):
                            h, mms = item
                            for kb, col0, avw, ex_t in mms:
                                nc.tensor.matmul(
                                    avs[h][0:65, col0:col0 + avw],
                                    v_aug[:, kb, h, :],
                                    ex_t,
                                    start=(kb == 0),
                                    stop=(kb == nkb - 1),
                                )

                        for blk in range(nblk):
                            kb0 = blk * pairw
                            diag = kb0 + pairw - 1 >= 4 * qmb
                            scs = {}
                            if wave == 2:
                                # fp8 DoubleRow scores: contraction hd=64
                                # folded as 32 partitions x 2 f-tiles.
                                for h in whs:
                                    scs[h] = ps_sc.tile(
                                        [128, W], F32, tag="sc",
                                        name=f"sc{h}"
                                    )
                                for sub in range(pairw):
                                    kb = kb0 + sub
                                    j = kb - 4 * qmb
                                    col0 = 128 * j if j >= 0 else 0
                                    cb = sub * 512 + col0
                                    scw = 512 - col0
                                    for h in whs:
                                        p0 = 32 * h
                                        nc.tensor.matmul(
                                            scs[h][:, cb:cb + scw],
                                            qkT[p0:p0 + 32, 0:2,
                                                kb * KB:(kb + 1) * KB],
                                            qkT[p0:p0 + 32, 2:4,
                                                q0 + col0:q0 + col0 + scw],
                                            start=True,
                                            stop=(j < 0),
                                            perf_mode=DR,
                                            skip_group_check=True,
                                            tile_position=(p0, 0),
                                        )
                                if diag:
                                    for h in whs:
                                        for sub in range(pairw):
                                            j = kb0 + sub - 4 * qmb
                                            if j < 0:
                                                continue
                                            cb = sub * 512 + 128 * j
                                            nc.tensor.matmul(
                                                scs[h][:, cb:cb + 128],
                                                identz_dr,
                                                trimask_dr,
                                                start=False,
                                                stop=True,
                                                perf_mode=DR,
                                                skip_group_check=True,
                                            )
                            for h in whs:
                                sc = scs[h]
                                ex = expp.tile([128, W], BF16)
                                if bmode == "tiny_exp":
                                    nc.scalar.activation(
                                        out=ex[:, 0:64],
                                        in_=sc[:, 0:64],
                                        func=mybir.ActivationFunctionType.Exp,
                                        scale=SCALE,
                                    )
                                elif pairw == 1 or (diag and not fullexp):
                                    for sub in range(pairw):
                                        j = kb0 + sub - 4 * qmb
                                        col0 = 128 * j if j >= 0 else 0
                                        cb = sub * 512 + col0
                                        nc.scalar.activation(
                                            out=ex[:, cb:sub * 512 + 512],
                                            in_=sc[:, cb:sub * 512 + 512],
                                            func=(mybir
                                                  .ActivationFunctionType.Exp),
                                            scale=SCALE,
                                        )
                                else:
                                    # one full-tile exp even for diagonal
                                    # pairs: the below-diagonal garbage
                                    # region of ex is never read by the av
                                    # matmuls (they slice [col0:512]).
                                    nc.scalar.activation(
                                        out=ex,
                                        in_=sc,
                                        func=mybir.ActivationFunctionType.Exp,
                                        scale=SCALE,
                                    )
                                mms = []
                                for sub in range(pairw):
                                    kb = kb0 + sub
                                    j = kb - 4 * qmb
                                    col0 = 128 * j if j >= 0 else 0
                                    avw = (64 if bmode == "tiny_av"
                                           else QMB - col0)
                                    mms.append((
                                        kb, col0, avw,
                                        ex[:, sub * 512 + col0:
                                            sub * 512 + col0 + avw],
                                    ))
                                avq.append((h, mms))
                            if fill_first and filler:
                                filler.pop(0)()
                            while len(avq) > wave * lag:
                                emit_av(avq.pop(0))
                            if not fill_first and filler:
                                filler.pop(0)()
                        while avq:
                            emit_av(avq.pop(0))

                        # normalize: values = av[0:64] / av[64]
                        for h in whs:
                            av = avs[h]
                            if bmode == "notail":
                                snk = small.tile([1, QMB], F32, tag="snk")
                                nc.vector.tensor_copy(snk, av[64:65, :])
                                dma.dma_start(
                                    out=out[128 + h:129 + h, 0:QMB], in_=snk
                                )
                                continue
                            rec = small.tile([1, QMB], F32R, tag="rec")
                            with nc.allow_low_precision(
                                reason="softmax denom feeds f32r matmul"
                            ):
                                nc.vector.reciprocal(rec, av[64:65, :])
                            rb = small.tile([64, QMB], F32R, tag="rb")
                            nc.gpsimd.partition_broadcast(rb, rec)
                            dt_ = h // 2
                            pr = 64 * (h % 2)
                            nc.vector.tensor_mul(
                                valuesT[pr:pr + 64, dt_, q0:q0 + QMB],
                                av[0:64, :],
                                rb,
                            )
                    # ---- queue stage C for this qmb ----
                    if "C" not in stages:
                        if bmode != "notail":
                            dma.dma_start(
                                out=out[qmb * 128:(qmb + 1) * 128, 0:512],
                                in_=valuesT[:, 0, qmb * 512:qmb * 512 + 512]
                                .bitcast(F32),
                            )
                        continue
                    for sti in range(QMB // 128):
                        filler.append(make_c_item(qmb * 4 + sti))
                while filler:
                    filler.pop(0)()

            if repeat == 1:
                body(0)
            else:
                with tc.For_i(
                    0, repeat, 1,
                    hint_engines=(mybir.EngineType.PE,),
                ) as it:
                    body(it)
    nc.compile()
    return nc


def make_in_maps(x, W_qkv, b_qkv, W_out, b_out):
    """Host-side sharding: per-core input dict."""
    import ml_dtypes
    bf16 = ml_dtypes.bfloat16
    fp8 = ml_dtypes.float8_e4m3
    x = np.asarray(x, dtype=np.float32)
    W_qkv = np.asarray(W_qkv, dtype=np.float32)
    b_qkv = np.asarray(b_qkv, dtype=np.float32)
    W_out = np.asarray(W_out, dtype=np.float32)
    in_maps = []
    xT_f32 = [np.ascontiguousarray(x[b_].T) for b_ in range(B)]
    xT_by_b = [np.ascontiguousarray(t.astype(bf16)) for t in xT_f32]
    xT8_by_b = [np.ascontiguousarray(t.astype(fp8)) for t in xT_f32]
    for c in range(N_CORES):
        b_ = c // 4
        g = c % 4
        heads = [4 * g + i for i in range(HL)]
        # feature tiles (DoubleRow layout): partition 32*hl+p = local head hl,
        # dim p; ft0 = K dims 0-31, ft1 = K dims 32-63, ft2 = Q lo, ft3 = Q hi
        qk_cols = []
        for kind, off in ((1, 0), (1, 32), (0, 0), (0, 32)):  # (K/Q, dim off)
            for h_ in heads:
                base = h_ * 3 * HD + kind * HD + off
                qk_cols.extend(range(base, base + 32))
        v_cols = []
        for h_ in heads:
            base = h_ * 3 * HD + 2 * HD  # V
            v_cols.extend(range(base, base + HD))
        qk_cols = np.array(qk_cols)
        v_cols = np.array(v_cols)
        in_maps.append({
            "xT": xT_by_b[b_],
            "xT8": xT8_by_b[b_],
            "wqk": np.ascontiguousarray(W_qkv[:, qk_cols].astype(fp8)),
            "wv": np.ascontiguousarray(W_qkv[:, v_cols].astype(bf16)),
            "wo": np.ascontiguousarray(W_out[g * FV:(g + 1) * FV, :].astype(bf16)),
            "bqk": np.ascontiguousarray(b_qkv[qk_cols]),
            "bv": np.ascontiguousarray(b_qkv[v_cols]),
        })
    return in_maps


_NC_CACHE = {}


def get_nc(repeat: int = 1):
    if repeat not in _NC_CACHE:
        _NC_CACHE[repeat] = build_kernel(repeat)
    return _NC_CACHE[repeat]


def kernel(x, W_qkv, b_qkv, W_out, b_out):
    in_maps = make_in_maps(x, W_qkv, b_qkv, W_out, b_out)
    nc = get_nc(1)
    res = run_bass_kernel_spmd(nc, in_maps, list(range(N_CORES)))
    b_out = np.asarray(b_out, dtype=np.float32)
    out = np.zeros((B, S, D), dtype=np.float32)
    for b_ in range(B):
        acc = np.zeros((S, D), dtype=np.float32)
        for g in range(4):
            acc += res.results[4 * b_ + g]["out"].astype(np.float32)
        out[b_] = acc + b_out[None, :]
    return out

